# revision 1
# baseline (speedup 1.0000x reference)
"""FPN ROIAlign pooler (nn_Pooler) on 8 trn2 cores.

Strategy: data-parallel over RoIs (boxes dealt round-robin by level-group for
balance). Host builds a channels-last pixel table [161508px, 256ch] fp32 and
per-core gather index/weight streams. Device: for each 128-bin chunk,
dma_gather 1024 rows (8 gathers x 128 bins; gather = one (y-row, x-window) of
one bilinear corner pair), multiply by per-bin weights (DVE broadcast), reduce
over the 8 x window taps -> [128 bins, 256 ch], DMA out. Host reassembles
[1024, 256, 7, 7].
"""
import numpy as np
from contextlib import ExitStack

from concourse import bacc, bass, mybir, tile, bass_utils

C = 256
N_CORES = 8
OUT = 7
LVL_HW = [(200, 304), (100, 152), (50, 76), (25, 38)]
SCALES = (0.25, 0.125, 0.0625, 0.03125)
SEG_SZ = [h * w for h, w in LVL_HW]           # px per (lvl, batch) segment
# segment order: (0,0),(0,1),(1,0),(1,1),(2,0),(2,1),(3,0),(3,1)
SEG_BASE = np.zeros((4, 2), np.int64)
_off = 0
for _l in range(4):
    for _b in range(2):
        SEG_BASE[_l, _b] = _off
        _off += SEG_SZ[_l]
TOTAL_PX = int(_off)                           # 161500
END_PAD_PX = 8
TABLE_PX = TOTAL_PX + END_PAD_PX

# gather groups: (base_px, step_px, win_px, n_rows)
GROUPS = [
    (0, 2, 3, 30400),          # lvl0 batch0 (2px-stride rows, 3px window)
    (60800, 2, 3, 30400),      # lvl0 batch1
    (121600, 1, 2, 30400),     # lvl1 both batches
    (152000, 1, 2, 9500),      # lvl2+lvl3 all
]
GRP_WIN = [3, 3, 2, 2]

NQ = 1024          # gather slots per chunk (8 taps x 128 bins)
IDXC = NQ // 16    # idx columns per chunk

_nc_cache = {}


def _build_nc(chunks):
    nc = bacc.Bacc("TRN2", target_bir_lowering=False, debug=False,
                   num_devices=N_CORES)
    nch = sum(chunks)
    wcols = sum(8 * GRP_WIN[g] * chunks[g] for g in range(4))
    table_d = nc.dram_tensor("table", [TABLE_PX * C], mybir.dt.float32,
                             kind="ExternalInput")
    idx_d = nc.dram_tensor("idxs", [128, IDXC * nch], mybir.dt.int16,
                           kind="ExternalInput")
    w_d = nc.dram_tensor("wts", [128, wcols], mybir.dt.float32,
                         kind="ExternalInput")
    out_d = nc.dram_tensor("out", [nch * 128, C], mybir.dt.float32,
                           kind="ExternalOutput")

    with tile.TileContext(nc) as tc, ExitStack() as ctx:
        sbi = ctx.enter_context(tc.tile_pool(name="sbi", bufs=3))
        sbd = ctx.enter_context(tc.tile_pool(name="sbd", bufs=2))
        sbp = ctx.enter_context(tc.tile_pool(name="sbp", bufs=2))
        sbo = ctx.enter_context(tc.tile_pool(name="sbo", bufs=2))

        ci = 0
        woff = 0
        for g in range(4):
            base_px, step_px, win_px, n_rows = GROUPS[g]
            es = win_px * C
            in_ap = bass.AP(tensor=table_d, offset=base_px * C,
                            ap=[[step_px * C, n_rows], [1, es]])
            for _ in range(chunks[g]):
                idx_t = sbi.tile([128, IDXC], mybir.dt.int16)
                nc.default_dma_engine.dma_start(
                    out=idx_t[:], in_=idx_d.ap()[:, ci * IDXC:(ci + 1) * IDXC])
                dst_t = sbd.tile([128, 8, es], mybir.dt.float32)
                nc.gpsimd.dma_gather(dst_t[:], in_ap, idx_t[:], NQ, NQ, es,
                                     elem_step=step_px * C)
                w_t = sbi.tile([128, 8, win_px], mybir.dt.float32)
                nc.default_dma_engine.dma_start(
                    out=w_t[:].rearrange("p a b -> p (a b)"),
                    in_=w_d.ap()[:, woff:woff + 8 * win_px])
                prod_t = sbp.tile([128, 8, win_px, C], mybir.dt.float32)
                nc.vector.tensor_tensor(
                    out=prod_t[:],
                    in0=dst_t[:].rearrange("p t (x c) -> p t x c", x=win_px, c=C),
                    in1=w_t[:].unsqueeze(3).broadcast_to([128, 8, win_px, C]),
                    op=mybir.AluOpType.mult)
                out_t = sbo.tile([128, C], mybir.dt.float32)
                nc.vector.tensor_reduce(
                    out=out_t[:], in_=prod_t[:].transpose([0, 3, 1, 2]),
                    axis=mybir.AxisListType.XY, op=mybir.AluOpType.add)
                nc.default_dma_engine.dma_start(
                    out=out_d.ap()[ci * 128:(ci + 1) * 128, :], in_=out_t[:])
                ci += 1
                woff += 8 * win_px
    nc.compile()
    return nc


def _host_prep(f0, f1, f2, f3, boxes, bidx):
    boxes32 = np.asarray(boxes, np.float32)
    b = np.asarray(bidx).astype(np.int64)
    N = boxes32.shape[0]

    # level routing in strict fp32 (matches jax reference arithmetic)
    x1, y1, x2, y2 = (boxes32[:, k] for k in range(4))
    area = (x2 - x1 + np.float32(1.0)) * (y2 - y1 + np.float32(1.0))
    s = np.sqrt(area)
    lv = np.floor(np.float32(4.0) + np.log2(s / np.float32(224.0)
                                            + np.float32(1e-6)))
    lvl = (np.clip(lv, 2.0, 5.0)).astype(np.int64) - 2

    # channels-last flat table
    segs = []
    for f in (f0, f1, f2, f3):
        fa = np.asarray(f, np.float32)
        for bb in range(2):
            segs.append(np.transpose(fa[bb], (1, 2, 0)).reshape(-1, C))
    segs.append(np.zeros((END_PAD_PX, C), np.float32))
    table_flat = np.ascontiguousarray(np.concatenate(segs, 0)).reshape(-1)

    scs = np.array(SCALES)[lvl]
    Wl = np.array([hw[1] for hw in LVL_HW])[lvl]
    Hl = np.array([hw[0] for hw in LVL_HW])[lvl]
    x1s = boxes32[:, 0].astype(np.float64) * scs
    y1s = boxes32[:, 1].astype(np.float64) * scs
    x2s = boxes32[:, 2].astype(np.float64) * scs
    y2s = boxes32[:, 3].astype(np.float64) * scs
    bin_w = np.maximum(x2s - x1s, 1.0) / OUT
    bin_h = np.maximum(y2s - y1s, 1.0) / OUT
    grid = (np.arange(OUT)[:, None] + np.array([0.25, 0.75])[None, :]).reshape(-1)
    xs = x1s[:, None] + bin_w[:, None] * grid[None, :]     # [N,14]
    ys = y1s[:, None] + bin_h[:, None] * grid[None, :]
    vx = (xs >= -1.0) & (xs <= Wl[:, None])
    vy = (ys >= -1.0) & (ys <= Hl[:, None])
    xc = np.clip(xs, 0.0, (Wl - 1)[:, None])
    yc = np.clip(ys, 0.0, (Hl - 1)[:, None])
    x0c = np.minimum(np.floor(xc).astype(np.int64), (Wl - 2)[:, None])
    y0c = np.minimum(np.floor(yc).astype(np.int64), (Hl - 2)[:, None])
    lx = xc - x0c
    ly = yc - y0c

    seg_base = SEG_BASE[lvl, b]
    group = np.where(lvl == 0, b, np.where(lvl == 1, 2, 3))
    GRP_BASE_PX = np.array([0, 60800, 121600, 152000])
    gbase = GRP_BASE_PX[group]

    # addr[n, sy, t, sx]
    yrow = y0c[:, :, None] + np.arange(2)[None, None, :]            # [N,14,2]
    addr = (seg_base[:, None, None, None]
            + yrow[:, :, :, None] * Wl[:, None, None, None]
            + x0c[:, None, None, :])                                 # [N,14,2,14]
    local = addr - gbase[:, None, None, None]
    is0 = (lvl == 0)[:, None, None, None]
    row = np.where(is0, local // 2, local)
    par = np.where(is0, local % 2, 0)

    yw = np.stack([1.0 - ly, ly], axis=2)                            # [N,14,2]
    xw = np.stack([1.0 - lx, lx], axis=2)                            # [N,14,2]
    valid = (vy[:, :, None] & vx[:, None, :])                        # [N,14,14]
    base_w = valid[:, :, None, :] * yw[:, :, :, None] * 0.25         # [N,14,2,14]
    WMAX = 3
    wfull = np.zeros((N, 14, 2, 14, WMAX))
    for k in range(2):
        np.put_along_axis(
            wfull, (par + k)[..., None],
            (base_w * xw[:, None, None, :, k])[..., None], axis=4)

    # -> bins: sy=(by,iy), sx=(bx,ix); t8 = iy*4 + t*2 + ix
    idx_bin = (row.reshape(N, 7, 2, 2, 7, 2)
               .transpose(0, 1, 4, 2, 3, 5).reshape(N, 49, 8))
    w_bin = (wfull.reshape(N, 7, 2, 2, 7, 2, WMAX)
             .transpose(0, 1, 4, 2, 3, 5, 6).reshape(N, 49, 8, WMAX))

    for g in range(4):
        m = group == g
        if m.any():
            assert idx_bin[m].min() >= 0 and idx_bin[m].max() < GROUPS[g][3]

    # deal boxes: sort by group, core i takes sorted[i::8]
    order = np.argsort(group, kind="stable")
    core_boxes = [order[i::N_CORES] for i in range(N_CORES)]
    Bg = np.array([[np.sum(group[cb] == g) for g in range(4)]
                   for cb in core_boxes])
    maxb = Bg.max(axis=0)
    chunks = tuple(int(-(-49 * mb // 128)) for mb in maxb)

    idx_all, w_all, omap = [], [], []
    for core in range(N_CORES):
        cb = core_boxes[core]
        icols, wcols, cmap = [], [], []
        row_base = 0
        for g in range(4):
            bl = cb[group[cb] == g]
            nb = len(bl)
            tot = chunks[g] * 128
            win = GRP_WIN[g]
            ib = np.zeros((tot, 8), np.int64)
            wv = np.zeros((tot, 8, win))
            if nb:
                ib[:49 * nb] = idx_bin[bl].reshape(-1, 8)
                wv[:49 * nb] = w_bin[bl][..., :win].reshape(-1, 8, win)
            for c in range(chunks[g]):
                ic = ib[c * 128:(c + 1) * 128].T.reshape(NQ)     # slot=t8*128+q
                assert ic.max() < 32768
                wr = np.tile(ic.reshape(IDXC, 16).T.astype(np.int16), (8, 1))
                icols.append(wr)
                wcols.append(wv[c * 128:(c + 1) * 128]
                             .reshape(128, 8 * win).astype(np.float32))
            cmap.append((bl, row_base))
            row_base += tot
        idx_all.append(np.concatenate(icols, axis=1))
        w_all.append(np.concatenate(wcols, axis=1))
        omap.append(cmap)
    return table_flat, idx_all, w_all, omap, chunks


LAST_RESULT = None


def kernel(f0, f1, f2, f3, boxes, box_batch_idx):
    global LAST_RESULT
    table_flat, idx_all, w_all, omap, chunks = _host_prep(
        f0, f1, f2, f3, boxes, box_batch_idx)
    if chunks not in _nc_cache:
        _nc_cache[chunks] = _build_nc(chunks)
    nc = _nc_cache[chunks]
    in_maps = [{"table": table_flat, "idxs": idx_all[i], "wts": w_all[i]}
               for i in range(N_CORES)]
    res = bass_utils.run_bass_kernel_spmd(nc, in_maps,
                                          core_ids=list(range(N_CORES)))
    LAST_RESULT = res

    outfull = np.zeros((1024, 49, C), np.float32)
    for core in range(N_CORES):
        r = np.asarray(res.results[core]["out"])
        for (bl, row_base) in omap[core]:
            nb = len(bl)
            if nb:
                outfull[bl] = r[row_base:row_base + 49 * nb].reshape(nb, 49, C)
    return np.ascontiguousarray(
        outfull.transpose(0, 2, 1).reshape(1024, C, OUT, OUT))

